# revision 9
# baseline (speedup 1.0000x reference)
"""Multi-head self-attention (RoPE, causal) on 8 Trainium2 NeuronCores.

Problem: B=1, S=2048, D=1024, H=16 heads, d_k=64, causal, interleaved RoPE.

Sharding: tensor-parallel over heads for QKV+attention (2 heads/core),
then AllToAll to switch to sequence sharding, so the output projection
is fully local (each core computes y rows [256c:256c+256] for all 1024
output dims). Host concatenates row slices — no host arithmetic.

v2 design (vs the fp32r baseline):
  - fp16 data everywhere (inputs, Q/K/V, P, attn, WO weights); all
    matmul accumulation stays fp32 in PSUM.  Halves DMA + SBUF + the
    AllToAll payload (0.5 MB/rank), enables FWL fast weight loads and
    DVE 2x/4x 16-bit modes.  fp16 (10-bit mantissa) ~ fp32r precision.
  - Chunk-interleaved schedule: QKV projection + RoPE + V for q-chunk
    j+1 are emitted as "filler" units spread between the score/AV pairs
    of attention chunk j, so the PE never stalls waiting on ACT's exp
    and ACT's exp pipeline starts ~5 us into the pass instead of after
    the whole QKV phase (the baseline ran the phases back-to-back:
    34.6 + 76.9 + 8.0 us measured -> attention was a serial
    score->exp->mask->AV chain).
  - Engine rebalance: DVE (0.96 GHz) keeps only PSUM reads + fp16
    SBUF ops (4x mode); Pool takes the RoPE multiply/add and the
    softmax-reciprocal broadcast; ACT does exp + final y copies.
  - WO weights resident in SBUF (fp16, 16 KB/partition), loaded once
    during the attention phase instead of streamed every pass.

Device layouts (per core c, local heads h0=2c, h1=2c+1):
  xt   [128, 8, 2048]  x^T (d on partitions, d-tile, s) — replicated
  qt/kt [128, 2048]    Q^T/K^T rows: [h0-even(32) h0-odd(32) h1-e h1-o]
                       (RoPE pair-permutation folded into weight slices)
  v_sb [128, 2, 16, 65] V' tiles [V(64) | ones] per (head, k-tile)
                       (ones col -> softmax sums ride AV as out row 64)
  scores S^T [k,q] in PSUM -> exp on ACT -> P^T fp16 -> AV: V'^T P
  normalize: rec=1/sums (DVE), partition_broadcast (Pool), mul (DVE)
  A2A shards attn^T [128e, 256s] fp16 -> each core gets attnT[:, S_c]
  WO: y[s,m] accumulated over 8 e-tiles from resident fp16 weights.
"""

import math
import numpy as np

import concourse.bass as bass
import concourse.mybir as mybir
import concourse.tile as tile
from concourse import bacc
from concourse.bass_utils import run_bass_kernel_spmd

F32 = mybir.dt.float32
F16 = mybir.dt.float16
AF = mybir.ActivationFunctionType
ALU = mybir.AluOpType

S = 2048
D = 1024
H = 16
DK = 64
NCORES = 8
EC = D // NCORES          # 128 e-dims per core (2 heads)
SC = S // NCORES          # 256 s-rows per core after A2A
NQ = 512                  # q-chunk width
NJ = S // NQ              # 4 q-chunks
KT = S // 128             # 16 k-tiles
DT = D // 128             # 8 d-tiles
THETA = 10000.0

_PROGRAM = None

_HINTS = (mybir.EngineType.PE, mybir.EngineType.Activation,
          mybir.EngineType.DVE, mybir.EngineType.Pool,
          mybir.EngineType.SP)


def _build_program(reps=1, collective=True, loop_stages=("single",),
                   look=6, ptbufs=8, att_mode="full", do_wo=True,
                   do_stage_dma=True, mask_engine="mm", **_unused):
    nc = bacc.Bacc("TRN2", target_bir_lowering=False, debug=False,
                   num_devices=NCORES if collective else 1)

    # ---- DRAM I/O (all fp16 except the fp32 output) ----
    xt_d = nc.dram_tensor("xt", [128, DT, S], F16, kind="ExternalInput").ap()
    wqt_d = nc.dram_tensor("wqt", [128, DT, EC], F16,
                           kind="ExternalInput").ap()
    wkt_d = nc.dram_tensor("wkt", [128, DT, EC], F16,
                           kind="ExternalInput").ap()
    wvt_d = nc.dram_tensor("wvt", [128, DT, EC], F16,
                           kind="ExternalInput").ap()
    wot_d = nc.dram_tensor("wot", [128, DT, D], F16,
                           kind="ExternalInput").ap()
    ctab_d = nc.dram_tensor("ctab", [128, S], F16, kind="ExternalInput").ap()
    stab_d = nc.dram_tensor("stab", [128, S], F32, kind="ExternalInput").ap()
    pswap_d = nc.dram_tensor("pswap", [128, 128], F16,
                             kind="ExternalInput").ap()
    msk_d = nc.dram_tensor("msk01", [128, 128], F16, kind="ExternalInput").ap()
    id_d = nc.dram_tensor("id128", [128, 128], F16, kind="ExternalInput").ap()
    mneg_d = nc.dram_tensor("mneg", [128, 128], F16,
                            kind="ExternalInput").ap()
    y_d = nc.dram_tensor("y_out", [SC, D], F32, kind="ExternalOutput").ap()

    # internal DRAM for the collective (fp16 payload: 0.5 MB/rank)
    a2a_in = nc.dram_tensor("a2a_in", [NCORES, EC, SC], F16)
    a2a_out = nc.dram_tensor("a2a_out", [NCORES, EC, SC], F16)

    scale = 1.0 / math.sqrt(DK)

    with tile.TileContext(nc) as tc:
        with (
            tc.tile_pool(name="persist", bufs=1) as pp,
            tc.tile_pool(name="work", bufs=3) as wp,
            tc.tile_pool(name="pt_pool", bufs=ptbufs) as ptp,
            tc.tile_pool(name="psum", bufs=2, space="PSUM") as ps,
            tc.tile_pool(name="psum_att", bufs=2, space="PSUM") as psa,
        ):
            # ---- resident loads ----
            wqt = pp.tile([128, DT, EC], F16)
            wkt = pp.tile([128, DT, EC], F16)
            wvt = pp.tile([128, DT, EC], F16)
            pswap = pp.tile([128, 128], F16)
            msk01 = pp.tile([128, 128], F16)
            id128 = pp.tile([128, 128], F16)
            mneg = pp.tile([128, 128], F16)
            nc.sync.dma_start(wqt[:], wqt_d[:])
            nc.sync.dma_start(wkt[:], wkt_d[:])
            nc.sync.dma_start(wvt[:], wvt_d[:])
            nc.sync.dma_start(pswap[:], pswap_d[:])
            nc.sync.dma_start(msk01[:], msk_d[:])
            nc.sync.dma_start(id128[:], id_d[:])
            nc.sync.dma_start(mneg[:], mneg_d[:])

            v_sb = pp.tile([128, 2, KT, 65], F16)
            nc.gpsimd.memset(v_sb[:, :, :, 64], 1.0)  # softmax-sum column

            ctab = pp.tile([128, S], F16)
            stab = pp.tile([128, S], F32)
            xt = pp.tile([128, DT, S], F16)
            for j in range(NJ):
                csl = slice(NQ * j, NQ * (j + 1))
                nc.sync.dma_start(ctab[:, csl], ctab_d[:, csl])
                nc.sync.dma_start(stab[:, csl], stab_d[:, csl])
                nc.sync.dma_start(xt[:, :, csl], xt_d[:, :, csl])
            wot = pp.tile([128, DT, D], F16)
            nc.sync.dma_start(wot[:], wot_d[:])

            qt = pp.tile([128, S], F16)    # RoPE'd Q^T
            kt = pp.tile([128, S], F16)    # RoPE'd K^T
            attnT = pp.tile([128, S], F16)  # rows: h0 in 0:64, h1 in 64:128
            at3 = pp.tile([128, DT, SC], F16)

            # ---- stage pieces ----
            def proj_unit(j, wt):
                csl = slice(NQ * j, NQ * (j + 1))
                g_ps = ps.tile([128, NQ], F32, name="g_ps", tag="mm")
                for t in range(DT):
                    nc.tensor.matmul(g_ps[:], wt[:, t, :], xt[:, t, csl],
                                     start=(t == 0), stop=(t == DT - 1))
                graw = wp.tile([128, NQ], F16, name="graw", tag="graw")
                nc.vector.tensor_copy(graw[:], g_ps[:])
                # swap matmul reuses g's PSUM bank (g is dead after the
                # copy; start=True only clears has_written bits)
                nc.tensor.matmul(g_ps[:], pswap[:], graw[:],
                                 start=True, stop=True)
                return graw, g_ps

            def rope_unit(j, graw, gsw_ps, out_sb):
                # rot = g*ctab + swap(g)*stab; multiply/add split so Pool
                # takes the fp16 SBUF-only ops and DVE the PSUM read.
                csl = slice(NQ * j, NQ * (j + 1))
                a_sb = wp.tile([128, NQ], F16, name="a_sb", tag="a_sb")
                nc.gpsimd.tensor_mul(a_sb[:], graw[:], ctab[:, csl])
                b_sb = wp.tile([128, NQ], F16, name="b_sb", tag="b_sb")
                nc.vector.tensor_mul(b_sb[:], gsw_ps[:], stab[:, csl])
                nc.gpsimd.tensor_add(out_sb[:, csl], a_sb[:], b_sb[:])

            def v_unit(j, half, state):
                # two accumulation chains packed into quarters of one
                # shared PSUM bank per chunk, one strided copy per half
                if half == 0:
                    state["v_ps"] = ps.tile([128, NQ], F32, name="v_ps",
                                            tag="mm")
                v_ps = state["v_ps"]
                for u in range(2):
                    q = 2 * half + u
                    st = 4 * j + q
                    for t in range(DT):
                        nc.tensor.matmul(
                            v_ps[:, 128 * q:128 * (q + 1)],
                            xt[:, t, 128 * st:128 * (st + 1)],
                            wvt[:, t, :],
                            start=(t == 0), stop=(t == DT - 1))
                st0 = 4 * j + 2 * half
                nc.vector.tensor_copy(
                    v_sb[:, :, st0:st0 + 2, 0:64],
                    v_ps[:, 256 * half:256 * (half + 1)]
                    .rearrange("p (q h e) -> p h q e", q=2, h=2))

            def qk_units(j):
                state = {}

                def pq():
                    state["q"] = proj_unit(j, wqt)

                def rq():
                    rope_unit(j, *state["q"], qt)

                def pk():
                    state["k"] = proj_unit(j, wkt)

                def rk():
                    rope_unit(j, *state["k"], kt)

                units = [pq, rq, pk, rk]
                vstate = {}
                units += [(lambda half=half: v_unit(j, half, vstate))
                          for half in range(2)]
                return units

            def att_chunk(j, fillers):
                # k-tiles in pairs per head: two score MMs -> one exp ->
                # (diag masks) -> two AV MMs, with `look` pairs of
                # lookahead and filler units from chunk j+1 spread in.
                csl_q = slice(NQ * j, NQ * (j + 1))
                av_ps = [psa.tile([65, NQ], F32, name=f"av_ps{h}",
                                  tag="av", bufs=2) for h in range(2)]
                ndiag = 4 * j
                pairs = [(p, h) for p in range((ndiag + 4) // 2)
                         for h in range(2)]
                n = len(pairs)
                nf = len(fillers)
                pend = {}
                fi = 0

                def emit_score(k):
                    p, h = pairs[k]
                    i0 = 2 * p
                    rs = [i0 - ndiag, i0 + 1 - ndiag]
                    offs = [128 * r if r > 0 else 0 for r in rs]
                    ws = [NQ - o for o in offs]
                    cs = [0, ws[0]]
                    wtot = ws[0] + ws[1]
                    hs = slice(64 * h, 64 * (h + 1))
                    st2 = psa.tile([128, 2 * NQ], F32, name=f"st2{h}",
                                   tag="st2", bufs=2)
                    for q in range(2):
                        nc.tensor.matmul(
                            st2[:, cs[q]:cs[q] + ws[q]],
                            kt[hs, 128 * (i0 + q):128 * (i0 + q + 1)],
                            qt[hs, NQ * j + offs[q]:NQ * (j + 1)],
                            start=True, stop=True,
                            tile_position=(64 * h, 0))
                        if rs[0] >= 0 and mask_engine == "mm":
                            # causal mask folded into the scores: add -6e4
                            # to the diagonal 128-block (exp -> exact 0),
                            # keeping exp->AV free of mask ops
                            nc.tensor.matmul(
                                st2[:, cs[q]:cs[q] + 128],
                                id128[:], mneg[:],
                                start=False, stop=True)
                    pt = ptp.tile([128, 2 * NQ], F16, name="pt")
                    if att_mode != "scores":
                        nc.scalar.activation(pt[:, :wtot], st2[:, :wtot],
                                             AF.Exp, scale=scale)
                        if rs[0] >= 0 and mask_engine in ("dve", "pool"):
                            eng = (nc.vector if mask_engine == "dve"
                                   else nc.gpsimd)
                            for q in range(2):
                                eng.tensor_mul(
                                    pt[:, cs[q]:cs[q] + 128],
                                    pt[:, cs[q]:cs[q] + 128],
                                    msk01[:])
                    pend[k] = (pt, i0, offs, ws, cs)

                def emit_av(k):
                    p, h = pairs[k]
                    pt, i0, offs, ws, cs = pend.pop(k)
                    if att_mode == "scores":
                        return
                    for q in range(2):
                        ii = i0 + q
                        nc.tensor.matmul(
                            av_ps[h][:, offs[q]:],
                            v_sb[:, h, ii, :],
                            pt[:, cs[q]:cs[q] + ws[q]],
                            start=(ii == 0), stop=(ii == ndiag + 3))

                for k in range(n + look):
                    if k < n:
                        emit_score(k)
                        # only between head-pairs (odd k), so h0/h1 score
                        # MMs stay adjacent and overlap via tile_position
                        if k % 2 == 1:
                            while fi < nf and fi * n < (k + 1) * nf:
                                fillers[fi]()
                                fi += 1
                    if k >= look:
                        emit_av(k - look)
                while fi < nf:
                    fillers[fi]()
                    fi += 1

                for h in range(2):
                    if att_mode == "scores":
                        break
                    rec = wp.tile([1, NQ], F32, name="rec", tag="rec")
                    nc.vector.reciprocal(rec[:], av_ps[h][64:65, :])
                    bc = wp.tile([64, NQ], F32, name="bc", tag="bc")
                    nc.gpsimd.partition_broadcast(bc[:], rec[:])
                    nc.vector.tensor_mul(
                        attnT[64 * h:64 * (h + 1), csl_q],
                        av_ps[h][0:64, :], bc[:])
                if do_stage_dma and att_mode != "scores":
                    nc.sync.dma_start(
                        a2a_in.ap()[2 * j:2 * j + 2].transpose([1, 0, 2]),
                        attnT[:, 2 * SC * j:2 * SC * (j + 1)]
                        .rearrange("p (r c) -> p r c", r=2))

            def emit_main():
                for u in qk_units(0):
                    u()
                for j in range(NJ):
                    fill = qk_units(j + 1) if j + 1 < NJ else []
                    if att_mode == "none":
                        for u in fill:
                            u()
                    else:
                        att_chunk(j, fill)

            def emit_wo():
                nc.sync.dma_start(at3[:], a2a_out.ap().transpose([1, 0, 2]))
                y_d3 = y_d.rearrange("(sub p) m -> p sub m", sub=2)
                for nn in range(2):          # m-chunks of 512
                    y_ps = [ps.tile([128, 512], F32, name=f"y_ps{sub}",
                                    tag="mm") for sub in range(2)]
                    for t in range(DT):
                        for sub in range(2):
                            nc.tensor.matmul(
                                y_ps[sub][:],
                                at3[:, t, 128 * sub:128 * (sub + 1)],
                                wot[:, t, 512 * nn:512 * (nn + 1)],
                                start=(t == 0), stop=(t == DT - 1))
                    y_sb = wp.tile([128, 2, 512], F32, name="y_sb",
                                   tag="y_sb")
                    for sub in range(2):
                        nc.scalar.copy(y_sb[:, sub, :], y_ps[sub][:])
                    nc.sync.dma_start(
                        y_d3[:, :, 512 * nn:512 * (nn + 1)], y_sb[:])

            def emit_collective():
                nc.gpsimd.collective_compute(
                    "AllToAll", ALU.bypass,
                    replica_groups=[list(range(NCORES))],
                    ins=[a2a_in.ap().opt()],
                    outs=[a2a_out.ap().opt()],
                )

            if reps > 1:
                # Timing build: collective can't sit in a hardware loop;
                # wo reads stale a2a_out (timing-representative only).
                with tc.For_i(0, reps, 1, hint_engines=_HINTS):
                    emit_main()
                    if do_wo:
                        emit_wo()
                if collective:
                    emit_collective()
            else:
                emit_main()
                if collective:
                    emit_collective()
                emit_wo()

    nc.compile()
    return nc


def _get_program():
    global _PROGRAM
    if _PROGRAM is None:
        _PROGRAM = _build_program()
    return _PROGRAM


def _host_prep(x, token_positions, WQ, WK, WV, WO):
    x = np.asarray(x, dtype=np.float32)
    WQ = np.asarray(WQ, dtype=np.float32)
    WK = np.asarray(WK, dtype=np.float32)
    WV = np.asarray(WV, dtype=np.float32)
    WO = np.asarray(WO, dtype=np.float32)
    pos = np.asarray(token_positions).reshape(-1).astype(np.float32)

    def part_major(a2d):  # [D, C] -> [128, DT, C]
        return np.ascontiguousarray(
            a2d.reshape(DT, 128, a2d.shape[1]).transpose(1, 0, 2))

    xt3 = part_major(x.reshape(S, D).T).astype(np.float16)       # [128,8,S]
    wot3 = part_major(np.ascontiguousarray(WO.T)).astype(np.float16)

    inv_freq = (1.0 / (THETA ** (np.arange(0, DK, 2, dtype=np.float32)
                                 / np.float32(DK)))).astype(np.float32)
    ang = pos[:, None] * inv_freq[None, :]                  # [S, 32] f32
    cos = np.cos(ang).astype(np.float32).T                  # [32, S]
    sin = np.sin(ang).astype(np.float32).T
    ctab = np.ascontiguousarray(np.tile(cos, (4, 1))).astype(np.float16)
    stab = np.ascontiguousarray(
        np.concatenate([-sin, sin, -sin, sin], axis=0)).astype(np.float32)

    pswap = np.zeros((128, 128), np.float16)
    for i in range(128):
        blk, o = divmod(i, 32)
        j = (blk ^ 1) * 32 + o
        pswap[j, i] = 1.0

    msk01 = (np.arange(128)[None, :] >= np.arange(128)[:, None]) \
        .astype(np.float16)                                 # keep f >= p
    id128 = np.eye(128, dtype=np.float16)
    mneg = np.where(msk01 > 0, np.float16(0), np.float16(-60000.0))

    perm = np.concatenate([np.arange(0, DK, 2), np.arange(1, DK, 2)])
    in_maps = []
    for c in range(NCORES):
        rows = np.concatenate([128 * c + 64 * l + perm for l in range(2)])
        wqt = part_major(np.ascontiguousarray(WQ[rows, :].T)) \
            .astype(np.float16)
        wkt = part_major(np.ascontiguousarray(WK[rows, :].T)) \
            .astype(np.float16)
        vrows = np.arange(128 * c, 128 * (c + 1))
        wvt = part_major(np.ascontiguousarray(WV[vrows, :].T)) \
            .astype(np.float16)
        in_maps.append({
            "xt": xt3, "wqt": wqt, "wkt": wkt, "wvt": wvt,
            "wot": wot3,
            "ctab": ctab, "stab": stab, "pswap": pswap,
            "msk01": msk01, "id128": id128, "mneg": mneg,
        })
    return in_maps


def kernel(x, token_positions, WQ, WK, WV, WO):
    in_maps = _host_prep(x, token_positions, WQ, WK, WV, WO)
    nc = _get_program()
    res = run_bass_kernel_spmd(nc, in_maps, list(range(NCORES)))
    y = np.concatenate([res.results[c]["y_out"] for c in range(NCORES)],
                       axis=0)
    return y.reshape(1, S, D).astype(np.float32)


# revision 17
# speedup vs baseline: 1.7603x; 1.7603x over previous
"""Multi-head self-attention (RoPE, causal) on 8 Trainium2 NeuronCores.

Problem: B=1, S=2048, D=1024, H=16 heads, d_k=64, causal, interleaved RoPE.

Sharding: tensor-parallel over heads for QKV+attention (2 heads/core),
then AllToAll to switch to sequence sharding, so the output projection
is fully local (each core computes y rows [256c:256c+256] for all 1024
output dims). Host concatenates row slices — no host arithmetic.

v2+ design (vs the fp32r baseline, measured 128.7 us/pass):
  - fp16 data everywhere (inputs, Q/K/V, P, attn, WO weights); all
    matmul accumulation stays fp32 in PSUM.  Halves DMA + SBUF + the
    AllToAll payload (0.5 MB/rank), enables FWL fast weight loads and
    DVE 2x/4x 16-bit modes.  fp16 (10-bit mantissa) ~ fp32r precision.
  - Chunk-interleaved schedule: QKV projection + RoPE + V for q-chunk
    j+1 are emitted as "filler" units spread between the score/AV pairs
    of attention chunk j (only after the h1 pair, keeping the h0/h1
    score matmuls adjacent so tile_position row groups overlap them).
    The baseline ran the phases back-to-back (34.6 + 76.9 + 8.0 us
    measured): attention was a serial score->exp->mask->AV chain.
  - The pass is cross-engine-latency bound, not throughput bound, so
    hops are minimized: RoPE runs entirely on DVE (fp16 4x mode for
    SBUF-only ops); the pswap matmul reuses g's PSUM bank; the four V
    accumulation chains of a chunk share one PSUM bank with a single
    strided copy out; softmax normalize copies av out to SBUF first
    (releasing the PSUM bank for the next chunk) and runs
    reciprocal+broadcast+multiply once per chunk for both heads
    (attnT stored [64, 2S], heads on columns).
  - WO weights resident in SBUF (fp16, 16 KB/partition); merged
    multi-dim-AP DMAs for staging/at/y (20 -> 7 instructions/pass).

Device layouts (per core c, local heads h0=2c, h1=2c+1):
  xt   [128, 8, 2048]  x^T (d on partitions, d-tile, s) — replicated
  qt/kt [128, 2048]    Q^T/K^T rows: [h0-even(32) h0-odd(32) h1-e h1-o]
                       (RoPE pair-permutation folded into weight slices)
  v_sb [128, 2, 16, 65] V' tiles [V(64) | ones] per (head, k-tile)
                       (ones col -> softmax sums ride AV as out row 64)
  scores S^T [k,q] in PSUM -> exp on ACT -> P^T fp16 -> AV: V'^T P
  attnT [64, 2*2048]   unnormalized attn^T * 1/sums, head h at col S*h
  A2A shards attn^T fp16 -> each core gets its 256 s-rows of all heads
  WO: y[s,m] accumulated over 8 e-tiles from resident fp16 weights.
"""

import math
import numpy as np

import concourse.bass as bass
import concourse.mybir as mybir
import concourse.tile as tile
from concourse import bacc
from concourse.bass_utils import run_bass_kernel_spmd

F32 = mybir.dt.float32
F16 = mybir.dt.float16
AF = mybir.ActivationFunctionType
ALU = mybir.AluOpType

S = 2048
D = 1024
H = 16
DK = 64
NCORES = 8
EC = D // NCORES          # 128 e-dims per core (2 heads)
SC = S // NCORES          # 256 s-rows per core after A2A
NQ = 512                  # q-chunk width
NJ = S // NQ              # 4 q-chunks
KT = S // 128             # 16 k-tiles
DT = D // 128             # 8 d-tiles
THETA = 10000.0

_PROGRAM = None

_HINTS = (mybir.EngineType.PE, mybir.EngineType.Activation,
          mybir.EngineType.DVE, mybir.EngineType.Pool,
          mybir.EngineType.SP)


def _build_program(reps=1, collective=True, loop_stages=("single",),
                   look=6, ptbufs=8, att_mode="full", do_wo=True,
                   do_stage_dma=True, mask_engine="dve", do_norm=True,
                   rope_dve=True, **_unused):
    nc = bacc.Bacc("TRN2", target_bir_lowering=False, debug=False,
                   num_devices=NCORES if collective else 1)

    # ---- DRAM I/O (all fp16 except the fp32 output) ----
    xt_d = nc.dram_tensor("xt", [128, DT, S], F16, kind="ExternalInput").ap()
    wqt_d = nc.dram_tensor("wqt", [128, DT, EC], F16,
                           kind="ExternalInput").ap()
    wkt_d = nc.dram_tensor("wkt", [128, DT, EC], F16,
                           kind="ExternalInput").ap()
    wvt_d = nc.dram_tensor("wvt", [128, DT, EC], F16,
                           kind="ExternalInput").ap()
    wot_d = nc.dram_tensor("wot", [128, DT, D], F16,
                           kind="ExternalInput").ap()
    ctab_d = nc.dram_tensor("ctab", [128, S], F16, kind="ExternalInput").ap()
    stab_d = nc.dram_tensor("stab", [128, S], F32, kind="ExternalInput").ap()
    pswap_d = nc.dram_tensor("pswap", [128, 128], F16,
                             kind="ExternalInput").ap()
    msk_d = nc.dram_tensor("msk01", [128, 128], F16, kind="ExternalInput").ap()
    id_d = nc.dram_tensor("id128", [128, 128], F16, kind="ExternalInput").ap()
    mneg_d = nc.dram_tensor("mneg", [128, 128], F16,
                            kind="ExternalInput").ap()
    y_d = nc.dram_tensor("y_out", [SC, D], F32, kind="ExternalOutput").ap()

    # internal DRAM for the collective (fp16 payload: 0.5 MB/rank)
    a2a_in = nc.dram_tensor("a2a_in", [NCORES, EC, SC], F16)
    a2a_out = nc.dram_tensor("a2a_out", [NCORES, EC, SC], F16)

    scale = 1.0 / math.sqrt(DK)

    with tile.TileContext(nc) as tc:
        with (
            tc.tile_pool(name="persist", bufs=1) as pp,
            tc.tile_pool(name="work", bufs=3) as wp,
            tc.tile_pool(name="pt_pool", bufs=ptbufs) as ptp,
            tc.tile_pool(name="psum", bufs=2, space="PSUM") as ps,
            tc.tile_pool(name="psum_att", bufs=2, space="PSUM") as psa,
        ):
            # ---- resident loads ----
            wqt = pp.tile([128, DT, EC], F16)
            wkt = pp.tile([128, DT, EC], F16)
            wvt = pp.tile([128, DT, EC], F16)
            pswap = pp.tile([128, 128], F16)
            msk01 = pp.tile([128, 128], F16)
            id128 = pp.tile([128, 128], F16)
            mneg = pp.tile([128, 128], F16)
            nc.sync.dma_start(wqt[:], wqt_d[:])
            nc.sync.dma_start(wkt[:], wkt_d[:])
            nc.sync.dma_start(wvt[:], wvt_d[:])
            nc.sync.dma_start(pswap[:], pswap_d[:])
            nc.sync.dma_start(msk01[:], msk_d[:])
            nc.sync.dma_start(id128[:], id_d[:])
            nc.sync.dma_start(mneg[:], mneg_d[:])

            v_sb = pp.tile([128, 2, KT, 65], F16)
            nc.gpsimd.memset(v_sb[:, :, :, 64], 1.0)  # softmax-sum column

            ctab = pp.tile([128, S], F16)
            stab = pp.tile([128, S], F32)
            xt = pp.tile([128, DT, S], F16)
            for j in range(NJ):
                csl = slice(NQ * j, NQ * (j + 1))
                nc.sync.dma_start(ctab[:, csl], ctab_d[:, csl])
                nc.sync.dma_start(stab[:, csl], stab_d[:, csl])
                nc.sync.dma_start(xt[:, :, csl], xt_d[:, :, csl])
            wot = pp.tile([128, DT, D], F16)
            nc.sync.dma_start(wot[:], wot_d[:])

            pt_z = None
            if att_mode == "noexp":
                pt_z = pp.tile([128, 2 * NQ], F16)
                nc.gpsimd.memset(pt_z[:], 0.0)
            qt = pp.tile([128, S], F16)    # RoPE'd Q^T
            kt = pp.tile([128, S], F16)    # RoPE'd K^T
            attnT = pp.tile([64, 2 * S], F16)  # cols: head h at S*h + s
            at3 = pp.tile([128, DT, SC], F16)

            # ---- stage pieces ----
            def proj_unit(j, wt):
                csl = slice(NQ * j, NQ * (j + 1))
                g_ps = ps.tile([128, NQ], F32, name="g_ps", tag="mm")
                for t in range(DT):
                    nc.tensor.matmul(g_ps[:], wt[:, t, :], xt[:, t, csl],
                                     start=(t == 0), stop=(t == DT - 1))
                graw = wp.tile([128, NQ], F16, name="graw", tag="graw")
                nc.vector.tensor_copy(graw[:], g_ps[:])
                # swap matmul reuses g's PSUM bank (g is dead after the
                # copy; start=True only clears has_written bits)
                nc.tensor.matmul(g_ps[:], pswap[:], graw[:],
                                 start=True, stop=True)
                return graw, g_ps

            def rope_unit(j, graw, gsw_ps, out_sb):
                # rot = g*ctab + swap(g)*stab; multiply/add split so Pool
                # takes the fp16 SBUF-only ops and DVE the PSUM read.
                csl = slice(NQ * j, NQ * (j + 1))
                eng = nc.vector if rope_dve else nc.gpsimd
                a_sb = wp.tile([128, NQ], F16, name="a_sb", tag="a_sb")
                eng.tensor_mul(a_sb[:], graw[:], ctab[:, csl])
                b_sb = wp.tile([128, NQ], F16, name="b_sb", tag="b_sb")
                nc.vector.tensor_mul(b_sb[:], gsw_ps[:], stab[:, csl])
                eng.tensor_add(out_sb[:, csl], a_sb[:], b_sb[:])

            def v_unit(j, half, state):
                # two accumulation chains packed into quarters of one
                # shared PSUM bank per chunk, one strided copy per half
                if half == 0:
                    state["v_ps"] = ps.tile([128, NQ], F32, name="v_ps",
                                            tag="mm")
                v_ps = state["v_ps"]
                for u in range(2):
                    q = 2 * half + u
                    st = 4 * j + q
                    for t in range(DT):
                        nc.tensor.matmul(
                            v_ps[:, 128 * q:128 * (q + 1)],
                            xt[:, t, 128 * st:128 * (st + 1)],
                            wvt[:, t, :],
                            start=(t == 0), stop=(t == DT - 1))
                st0 = 4 * j + 2 * half
                nc.vector.tensor_copy(
                    v_sb[:, :, st0:st0 + 2, 0:64],
                    v_ps[:, 256 * half:256 * (half + 1)]
                    .rearrange("p (q h e) -> p h q e", q=2, h=2))

            def qk_units(j):
                state = {}

                def pq():
                    state["q"] = proj_unit(j, wqt)

                def rq():
                    rope_unit(j, *state["q"], qt)

                def pk():
                    state["k"] = proj_unit(j, wkt)

                def rk():
                    rope_unit(j, *state["k"], kt)

                units = [pq, rq, pk, rk]
                vstate = {}
                units += [(lambda half=half: v_unit(j, half, vstate))
                          for half in range(2)]
                return units

            def att_chunk(j, fillers):
                # k-tiles in pairs per head: two score MMs -> one exp ->
                # (diag masks) -> two AV MMs, with `look` pairs of
                # lookahead and filler units from chunk j+1 spread in.
                csl_q = slice(NQ * j, NQ * (j + 1))
                av_ps = [psa.tile([65, NQ], F32, name=f"av_ps{h}",
                                  tag="av", bufs=2) for h in range(2)]
                ndiag = 4 * j
                pairs = [(p, h) for p in range((ndiag + 4) // 2)
                         for h in range(2)]
                n = len(pairs)
                nf = len(fillers)
                pend = {}
                fi = 0

                def emit_score(k):
                    p, h = pairs[k]
                    i0 = 2 * p
                    rs = [i0 - ndiag, i0 + 1 - ndiag]
                    offs = [128 * r if r > 0 else 0 for r in rs]
                    ws = [NQ - o for o in offs]
                    cs = [0, ws[0]]
                    wtot = ws[0] + ws[1]
                    hs = slice(64 * h, 64 * (h + 1))
                    st2 = psa.tile([128, 2 * NQ], F32, name=f"st2{h}",
                                   tag="st2", bufs=2)
                    for q in range(2):
                        nc.tensor.matmul(
                            st2[:, cs[q]:cs[q] + ws[q]],
                            kt[hs, 128 * (i0 + q):128 * (i0 + q + 1)],
                            qt[hs, NQ * j + offs[q]:NQ * (j + 1)],
                            start=True, stop=True,
                            tile_position=(64 * h, 0))
                        if rs[0] >= 0 and mask_engine == "mm":
                            # causal mask folded into the scores: add -6e4
                            # to the diagonal 128-block (exp -> exact 0),
                            # keeping exp->AV free of mask ops
                            nc.tensor.matmul(
                                st2[:, cs[q]:cs[q] + 128],
                                id128[:], mneg[:],
                                start=False, stop=True)
                    pt = (pt_z if att_mode == "noexp" else
                          ptp.tile([128, 2 * NQ], F16, name="pt"))
                    if att_mode not in ("scores", "noexp"):
                        nc.scalar.activation(pt[:, :wtot], st2[:, :wtot],
                                             AF.Exp, scale=scale)
                        if rs[0] >= 0 and mask_engine in ("dve", "pool"):
                            eng = (nc.vector if mask_engine == "dve"
                                   else nc.gpsimd)
                            for q in range(2):
                                eng.tensor_mul(
                                    pt[:, cs[q]:cs[q] + 128],
                                    pt[:, cs[q]:cs[q] + 128],
                                    msk01[:])
                    pend[k] = (pt, i0, offs, ws, cs)

                def emit_av(k):
                    p, h = pairs[k]
                    pt, i0, offs, ws, cs = pend.pop(k)
                    if att_mode == "scores":
                        return
                    for q in range(2):
                        ii = i0 + q
                        nc.tensor.matmul(
                            av_ps[h][:, offs[q]:],
                            v_sb[:, h, ii, :],
                            pt[:, cs[q]:cs[q] + ws[q]],
                            start=(ii == 0), stop=(ii == ndiag + 3))

                for k in range(n + look):
                    if k < n:
                        emit_score(k)
                        # only between head-pairs (odd k), so h0/h1 score
                        # MMs stay adjacent and overlap via tile_position
                        if k % 2 == 1:
                            while fi < nf and fi * n < (k + 1) * nf:
                                fillers[fi]()
                                fi += 1
                    if k >= look:
                        emit_av(k - look)
                while fi < nf:
                    fillers[fi]()
                    fi += 1

                if att_mode != "scores" and do_norm:
                    # copy releases the av PSUM banks fast; the rest of the
                    # chain runs once per chunk on fp16 SBUF for both heads
                    avu = wp.tile([65, 2, NQ], F32, name="avu", tag="avu")
                    for h in range(2):
                        nc.vector.tensor_copy(avu[:, h, :], av_ps[h][:])
                    rec = wp.tile([1, 2, NQ], F32, name="rec", tag="rec")
                    nc.vector.reciprocal(rec[:], avu[64:65, :, :])
                    bc = wp.tile([64, 2, NQ], F32, name="bc", tag="bc")
                    nc.gpsimd.partition_broadcast(bc[:], rec[:])
                    nc.vector.tensor_mul(
                        attnT[:, :].rearrange("e (h s) -> e h s", h=2)
                        [:, :, NQ * j:NQ * (j + 1)],
                        avu[0:64, :, :], bc[:])
                if do_stage_dma and att_mode != "scores":
                    for r in (2 * j, 2 * j + 1):
                        nc.sync.dma_start(
                            a2a_in.ap()[r].rearrange("(h e) c -> e h c",
                                                     h=2),
                            attnT[:, :].rearrange("e (h s) -> e h s", h=2)
                            [:, :, SC * r:SC * (r + 1)])

            def emit_main():
                for u in qk_units(0):
                    u()
                for j in range(NJ):
                    fill = qk_units(j + 1) if j + 1 < NJ else []
                    if att_mode == "none":
                        for u in fill:
                            u()
                    else:
                        att_chunk(j, fill)

            def emit_wo():
                nc.sync.dma_start(at3[:], a2a_out.ap().transpose([1, 0, 2]))
                y_d3 = y_d.rearrange("(sub p) m -> p sub m", sub=2)
                for nn in range(2):          # m-chunks of 512
                    y_ps = [ps.tile([128, 512], F32, name=f"y_ps{sub}",
                                    tag="mm") for sub in range(2)]
                    for t in range(DT):
                        for sub in range(2):
                            nc.tensor.matmul(
                                y_ps[sub][:],
                                at3[:, t, 128 * sub:128 * (sub + 1)],
                                wot[:, t, 512 * nn:512 * (nn + 1)],
                                start=(t == 0), stop=(t == DT - 1))
                    y_sb = wp.tile([128, 2, 512], F32, name="y_sb",
                                   tag="y_sb")
                    for sub in range(2):
                        nc.scalar.copy(y_sb[:, sub, :], y_ps[sub][:])
                    nc.sync.dma_start(
                        y_d3[:, :, 512 * nn:512 * (nn + 1)], y_sb[:])

            def emit_collective():
                nc.gpsimd.collective_compute(
                    "AllToAll", ALU.bypass,
                    replica_groups=[list(range(NCORES))],
                    ins=[a2a_in.ap().opt()],
                    outs=[a2a_out.ap().opt()],
                )

            if reps > 1:
                # Timing build: collective can't sit in a hardware loop;
                # wo reads stale a2a_out (timing-representative only).
                with tc.For_i(0, reps, 1, hint_engines=_HINTS):
                    emit_main()
                    if do_wo:
                        emit_wo()
                if collective:
                    emit_collective()
            else:
                emit_main()
                if collective:
                    emit_collective()
                emit_wo()

    nc.compile()
    return nc


def _get_program():
    global _PROGRAM
    if _PROGRAM is None:
        _PROGRAM = _build_program()
    return _PROGRAM


def _host_prep(x, token_positions, WQ, WK, WV, WO):
    x = np.asarray(x, dtype=np.float32)
    WQ = np.asarray(WQ, dtype=np.float32)
    WK = np.asarray(WK, dtype=np.float32)
    WV = np.asarray(WV, dtype=np.float32)
    WO = np.asarray(WO, dtype=np.float32)
    pos = np.asarray(token_positions).reshape(-1).astype(np.float32)

    def part_major(a2d):  # [D, C] -> [128, DT, C]
        return np.ascontiguousarray(
            a2d.reshape(DT, 128, a2d.shape[1]).transpose(1, 0, 2))

    xt3 = part_major(x.reshape(S, D).T).astype(np.float16)       # [128,8,S]
    wot3 = part_major(np.ascontiguousarray(WO.T)).astype(np.float16)

    inv_freq = (1.0 / (THETA ** (np.arange(0, DK, 2, dtype=np.float32)
                                 / np.float32(DK)))).astype(np.float32)
    ang = pos[:, None] * inv_freq[None, :]                  # [S, 32] f32
    cos = np.cos(ang).astype(np.float32).T                  # [32, S]
    sin = np.sin(ang).astype(np.float32).T
    ctab = np.ascontiguousarray(np.tile(cos, (4, 1))).astype(np.float16)
    stab = np.ascontiguousarray(
        np.concatenate([-sin, sin, -sin, sin], axis=0)).astype(np.float32)

    pswap = np.zeros((128, 128), np.float16)
    for i in range(128):
        blk, o = divmod(i, 32)
        j = (blk ^ 1) * 32 + o
        pswap[j, i] = 1.0

    msk01 = (np.arange(128)[None, :] >= np.arange(128)[:, None]) \
        .astype(np.float16)                                 # keep f >= p
    id128 = np.eye(128, dtype=np.float16)
    mneg = np.where(msk01 > 0, np.float16(0), np.float16(-60000.0))

    perm = np.concatenate([np.arange(0, DK, 2), np.arange(1, DK, 2)])
    in_maps = []
    for c in range(NCORES):
        rows = np.concatenate([128 * c + 64 * l + perm for l in range(2)])
        wqt = part_major(np.ascontiguousarray(WQ[rows, :].T)) \
            .astype(np.float16)
        wkt = part_major(np.ascontiguousarray(WK[rows, :].T)) \
            .astype(np.float16)
        vrows = np.arange(128 * c, 128 * (c + 1))
        wvt = part_major(np.ascontiguousarray(WV[vrows, :].T)) \
            .astype(np.float16)
        in_maps.append({
            "xt": xt3, "wqt": wqt, "wkt": wkt, "wvt": wvt,
            "wot": wot3,
            "ctab": ctab, "stab": stab, "pswap": pswap,
            "msk01": msk01, "id128": id128, "mneg": mneg,
        })
    return in_maps


def kernel(x, token_positions, WQ, WK, WV, WO):
    in_maps = _host_prep(x, token_positions, WQ, WK, WV, WO)
    nc = _get_program()
    res = run_bass_kernel_spmd(nc, in_maps, list(range(NCORES)))
    y = np.concatenate([res.results[c]["y_out"] for c in range(NCORES)],
                       axis=0)
    return y.reshape(1, S, D).astype(np.float32)
